# revision 1
# baseline (speedup 1.0000x reference)
"""DD-RoPE kernel for 8x TRN2 NeuronCores.

Reference computation (B=4, T=4096, D=2048, P=256):
    deltas = einsum('btd,pd->btp', x, W) + b     # (B, T, P)
    angles = cumsum(deltas, axis=1)
    out = concat([x1*cos(a) - x2*sin(a), x2*cos(a) + x1*sin(a), x[..., 512:]], -1)

Sharding: 8 shards = 4 batches x 2 T-halves (2048 each), data-parallel.
The cumsum is handled with host-computed fp64 "block bases": the exact
cumulative angle at every 128-step boundary (one [256, 16] vector set per
shard, computed from 128-step block sums of x in one pass). Each on-device
prefix scan then only spans 128 steps, so per-delta rounding error from the
reduced-precision matmul amplifies by at most sqrt(128), and there is no
cross-core (or even cross-block) dependency at all.

Per-core dataflow (all tensors in [feature-partition, time-free] layout):
    xf fp16 = fp16(x_shard^T), one dense 2MB DMA per 512-step time block
              (d-chunks side by side in the free dim of one SBUF tile)
    deltas^T = wh_f16^T @ xf + wlo_bf16^T @ xf + b_hi + b_lo
               (PE: 2 passes, mixed-dtype second pass, fp32 PSUM;
                split precision is needed because the cumsum amplifies
                per-delta error by sqrt(block))
    angles^T = per-128 prefix scans of deltas^T, initial = host base
    range-reduce in turns (magic-number rounding), sin/cos via ScalarE Sin
    rotation on DVE (o1) + GpSimd (o2), written into one output tile per
    time block -> one dense 1MB DMA out
    host reassembles the rotated half; passthrough cols copied on host.
"""

import sys

if "/opt/trn_rl_repo" not in sys.path:
    sys.path.insert(0, "/opt/trn_rl_repo")

from contextlib import ExitStack

import ml_dtypes
import numpy as np

import concourse.bacc as bacc
import concourse.bass as bass
import concourse.mybir as mybir
import concourse.tile as tile
from concourse.bass_utils import run_bass_kernel_spmd

F32 = mybir.dt.float32
F16 = mybir.dt.float16
BF16 = mybir.dt.bfloat16
ADD = mybir.AluOpType.add
SUB = mybir.AluOpType.subtract
IDENT = mybir.ActivationFunctionType.Identity
SIN = mybir.ActivationFunctionType.Sin

D = 2048          # input feature dim (contraction)
P = 256           # delta-pairs dim
ROT = 2 * P       # rotated columns (512)
TL = 2048         # time steps per shard
TB = 512          # time block (one PSUM bank at fp32)
SB = 128          # scan block (base injection granularity)
NT = TL // TB     # time blocks per shard (4)
NBK = TL // SB    # scan blocks per shard (16)
KC = D // 128     # contraction chunks (16)
N_CORES = 8

MAGIC = 12582912.0          # 1.5 * 2**23: fp32 round-to-int magic constant
SCALE_2PI = 6.28310         # slightly < 2*pi so Sin args stay inside [-pi, pi]
COS_BIAS = 1.5707964        # ~pi/2 (fp32)
NP_BF16 = np.dtype(ml_dtypes.bfloat16)


def build_program(tl: int = TL) -> bass.Bass:
    nt = tl // TB
    nbk = tl // SB
    nc = bacc.Bacc("TRN2", target_bir_lowering=False, debug=False)

    # Host-pre-tiled inputs: every DMA below reads one dense DRAM block.
    # xf row block tb: [128, KC*TB] fp16 (d-chunks along the free dim)
    xf = nc.dram_tensor("xf", [nt * 128, KC * TB], F16,
                        kind="ExternalInput").ap()
    wh = nc.dram_tensor("wh", [128, KC * P], F16, kind="ExternalInput").ap()
    wl = nc.dram_tensor("wl", [128, KC * P], BF16, kind="ExternalInput").ap()
    bv = nc.dram_tensor("bv", [1, 2 * P], BF16, kind="ExternalInput").ap()
    # per-128-block angle bases (turns), [P, nbk] fp32
    bs = nc.dram_tensor("bs", [P, nbk], F32, kind="ExternalInput").ap()
    # out row block tb: [128, 4*TB] f32 (quadrants o1h0|o1h1|o2h0|o2h1)
    outT = nc.dram_tensor("outT", [nt * 128, 4 * TB], F32,
                          kind="ExternalOutput").ap()

    with tile.TileContext(nc) as tc, ExitStack() as ctx:
        const_pool = ctx.enter_context(tc.tile_pool(name="const", bufs=1))
        w_pool = ctx.enter_context(tc.tile_pool(name="w", bufs=1))
        x_pool = ctx.enter_context(tc.tile_pool(name="x", bufs=2))
        psum_pool = ctx.enter_context(tc.tile_pool(name="psum", bufs=4, space="PSUM"))
        ang_pool = ctx.enter_context(tc.tile_pool(name="ang", bufs=2))
        trig_pool = ctx.enter_context(tc.tile_pool(name="trig", bufs=2))
        rot_pool = ctx.enter_context(tc.tile_pool(name="rot", bufs=2))
        out_pool = ctx.enter_context(tc.tile_pool(name="out", bufs=2))

        # Weights (stationary): one dense DMA per precision level
        wh_sb = w_pool.tile([128, KC * P], F16, tag="wh")
        nc.sync.dma_start(wh_sb[:], wh[:])
        wl_sb = w_pool.tile([128, KC * P], BF16, tag="wl")
        nc.sync.dma_start(wl_sb[:], wl[:])
        bs_sb = const_pool.tile([128, 2 * nbk], F32, tag="bs")
        nc.sync.dma_start(bs_sb[:, 0:nbk], bs[0:128, :])
        nc.sync.dma_start(bs_sb[:, nbk:2 * nbk], bs[128:256, :])
        bv_sb = const_pool.tile([1, 2 * P], BF16, tag="bv")
        nc.sync.dma_start(bv_sb[:], bv[:])
        ones_sb = const_pool.tile([1, TB], BF16, tag="ones")
        nc.gpsimd.memset(ones_sb[:], 1.0)
        zeros_sb = const_pool.tile([128, SB], F32, tag="zeros")
        nc.gpsimd.memset(zeros_sb[:], 0.0)
        magic_sb = const_pool.tile([128, 1], F32, tag="magic")
        nc.gpsimd.memset(magic_sb[:], MAGIC)
        negq_sb = const_pool.tile([128, 1], F32, tag="negq")
        nc.gpsimd.memset(negq_sb[:], -0.25)
        cosb_sb = const_pool.tile([128, 1], F32, tag="cosb")
        nc.gpsimd.memset(cosb_sb[:], COS_BIAS)

        for tb in range(nt):
            # one dense 2MB x DMA per time block
            xall = x_pool.tile([128, KC * TB], F16, tag="xall")
            nc.sync.dma_start(xall[:], xf[tb * 128:(tb + 1) * 128, :])
            oall = out_pool.tile([128, 4 * TB], F32, tag="oall")

            for h in range(2):
                # deltas^T (+bias) in PSUM: b_hi + b_lo + wh@xf + wl@xf
                dp = psum_pool.tile([128, TB], F32, tag="dp")
                nc.tensor.matmul(dp[:], bv_sb[0:1, h * 128:(h + 1) * 128],
                                 ones_sb[:], start=True, stop=False)
                nc.tensor.matmul(dp[:], bv_sb[0:1, P + h * 128:P + (h + 1) * 128],
                                 ones_sb[:], start=False, stop=False)
                for d in range(KC):
                    ws = slice(d * P + h * 128, d * P + (h + 1) * 128)
                    xs = slice(d * TB, (d + 1) * TB)
                    nc.tensor.matmul(dp[:], wh_sb[:, ws], xall[:, xs],
                                     start=False, stop=False)
                    nc.tensor.matmul(dp[:], wl_sb[:, ws], xall[:, xs],
                                     start=False, stop=(d == KC - 1))

                # cumulative angle (turns): independent per-128 scans with
                # host-computed initial bases
                ang = ang_pool.tile([128, TB], F32, tag=f"ang{h}")
                for k in range(TB // SB):
                    kb = tb * (TB // SB) + k
                    cs = slice(k * SB, (k + 1) * SB)
                    nc.vector.tensor_tensor_scan(
                        ang[:, cs], dp[:, cs], zeros_sb[:],
                        initial=bs_sb[:, h * nbk + kb:h * nbk + kb + 1],
                        op0=ADD, op1=ADD)

                # range reduction (turns): rs = y - round(y) in [-0.5, 0.5]
                a_s = trig_pool.tile([128, TB], F32, tag="a_s")
                nc.scalar.activation(a_s[:], ang[:], IDENT,
                                     bias=magic_sb[:], scale=-1.0)
                rs = trig_pool.tile([128, TB], F32, tag="rs")
                nc.vector.scalar_tensor_tensor(rs[:], a_s[:], MAGIC, ang[:],
                                               op0=SUB, op1=ADD)
                sin_t = trig_pool.tile([128, TB], F32, tag="sin")
                nc.scalar.activation(sin_t[:], rs[:], SIN, scale=SCALE_2PI)

                # rc = y - round(y + 0.25) in [-0.75, 0.25];
                # cos(2pi*y) = sin(2pi*rc + pi/2)
                b1 = trig_pool.tile([128, TB], F32, tag="b1")
                nc.scalar.activation(b1[:], ang[:], IDENT,
                                     bias=negq_sb[:], scale=-1.0)
                ac = trig_pool.tile([128, TB], F32, tag="ac")
                nc.scalar.activation(ac[:], b1[:], IDENT, bias=magic_sb[:])
                rc = trig_pool.tile([128, TB], F32, tag="rc")
                nc.vector.scalar_tensor_tensor(rc[:], ac[:], MAGIC, ang[:],
                                               op0=SUB, op1=ADD)
                cos_t = trig_pool.tile([128, TB], F32, tag="cos")
                nc.scalar.activation(cos_t[:], rc[:], SIN,
                                     scale=SCALE_2PI, bias=cosb_sb[:])

                # rotation: x1^T = d-chunk h, x2^T = d-chunk 2+h of xall.
                # o1 on DVE, o2 on the otherwise idle GpSimd.
                x1s = xall[:, h * TB:(h + 1) * TB]
                x2s = xall[:, (2 + h) * TB:(3 + h) * TB]
                t1 = rot_pool.tile([128, TB], F32, tag="t1")
                nc.vector.tensor_mul(t1[:], x1s, cos_t[:])
                t2 = rot_pool.tile([128, TB], F32, tag="t2")
                nc.vector.tensor_mul(t2[:], x2s, sin_t[:])
                o1 = oall[:, h * TB:(h + 1) * TB]
                nc.vector.tensor_sub(o1, t1[:], t2[:])
                t3 = rot_pool.tile([128, TB], F32, tag="t3")
                nc.gpsimd.tensor_mul(t3[:], x2s, cos_t[:])
                t4 = rot_pool.tile([128, TB], F32, tag="t4")
                nc.gpsimd.tensor_mul(t4[:], x1s, sin_t[:])
                o2 = oall[:, (2 + h) * TB:(3 + h) * TB]
                nc.gpsimd.tensor_add(o2, t3[:], t4[:])

            nc.sync.dma_start(outT[tb * 128:(tb + 1) * 128, :], oall[:])

    nc.compile()
    return nc


_NC_CACHE: dict = {}


def _get_nc():
    if "nc" not in _NC_CACHE:
        _NC_CACHE["nc"] = build_program()
    return _NC_CACHE["nc"]


def _tile_x(xt16: np.ndarray, nt: int) -> np.ndarray:
    """[D, tl] fp16 -> [nt*128, KC*TB]: row block tb, d-chunks along free."""
    tl = xt16.shape[1]
    a = xt16.reshape(KC, 128, tl // TB, TB).transpose(2, 1, 0, 3)
    return np.ascontiguousarray(a.reshape((tl // TB) * 128, KC * TB))


def prepare_weights(W: np.ndarray, b: np.ndarray):
    inv2pi = 1.0 / (2.0 * np.pi)
    Wt = W.astype(np.float64).T * inv2pi                           # [D, P]
    bt = b.astype(np.float64) * inv2pi                             # [P]
    whf = Wt.astype(np.float16)
    wlo = (Wt - whf.astype(np.float64)).astype(NP_BF16)
    # [D, P] -> [128, KC*P] with d-chunks along free dim
    wh_in = np.ascontiguousarray(
        whf.reshape(KC, 128, P).transpose(1, 0, 2).reshape(128, KC * P))
    wl_in = np.ascontiguousarray(
        wlo.reshape(KC, 128, P).transpose(1, 0, 2).reshape(128, KC * P))
    bh = bt.astype(NP_BF16)
    bl = (bt - bh.astype(np.float64)).astype(NP_BF16)
    bv_in = np.ascontiguousarray(np.concatenate([bh, bl])[None, :])
    # device-effective weights/bias for the host base computation
    w_eff = whf.astype(np.float64) + wlo.astype(np.float64)
    b_eff = bh.astype(np.float64) + bl.astype(np.float64)
    return wh_in, wl_in, bv_in, w_eff, b_eff


def make_in_maps(x: np.ndarray, W: np.ndarray, b: np.ndarray):
    B = x.shape[0]
    wh_in, wl_in, bv_in, w_eff, b_eff = prepare_weights(W, b)

    # fp64 cumulative angle at every 128-step boundary, per batch (in turns):
    # one pass of 128-block sums over x, then a small [32, D] @ [D, P] matmul
    T = x.shape[1]
    nblk = T // SB                                                  # 32
    xblk = x.reshape(B, nblk, SB, D).sum(axis=2, dtype=np.float64)  # [B, 32, D]
    dblk = xblk @ w_eff + SB * b_eff                                # [B, 32, P]
    bases = np.zeros((B, nblk, P))
    np.cumsum(dblk[:, :-1], axis=1, out=bases[:, 1:])               # exclusive

    in_maps = []
    for c in range(N_CORES):
        bb, hh = c // 2, c % 2
        xt16 = x[bb, hh * TL:(hh + 1) * TL, :].T.astype(np.float16)
        bs_in = bases[bb, hh * NBK:(hh + 1) * NBK].T                # [P, NBK]
        in_maps.append({
            "xf": _tile_x(xt16, NT),
            "wh": wh_in,
            "wl": wl_in,
            "bv": bv_in,
            "bs": np.ascontiguousarray(bs_in.astype(np.float32)),
        })
    return in_maps


def assemble_output(x: np.ndarray, results) -> np.ndarray:
    B, T, Din = x.shape
    out = np.empty((B, T, Din), np.float32)
    out[:, :, ROT:] = x[:, :, ROT:]
    for c in range(N_CORES):
        bb, hh = c // 2, c % 2
        r = results[c]["outT"].reshape(NT, 128, 4, TB)
        # [tb, pp, q(oi,h), u] -> [t_local(tb,u), p(oi,h,pp)]
        blk = r.transpose(0, 3, 2, 1).reshape(TL, ROT)
        out[bb, hh * TL:(hh + 1) * TL, :ROT] = blk
    return out


def kernel(x: np.ndarray, W: np.ndarray, b: np.ndarray) -> np.ndarray:
    nc = _get_nc()
    in_maps = make_in_maps(x, W, b)
    res = run_bass_kernel_spmd(nc, in_maps, list(range(N_CORES)))
    return assemble_output(x, res.results)



# revision 15
# speedup vs baseline: 1.4636x; 1.4636x over previous
"""DD-RoPE kernel for 8x TRN2 NeuronCores — "t-on-partitions" design.

Reference computation (B=4, T=4096, D=2048, P=256):
    deltas = einsum('btd,pd->btp', x, W) + b     # (B, T, P)
    angles = cumsum(deltas, axis=1)
    out = concat([x1*cos(a) - x2*sin(a), x2*cos(a) + x1*sin(a), x[..., 512:]], -1)

Sharding: 8 shards = 4 batches x 2 T-halves (2048 steps each), data-parallel.
The cumsum is split into independent 128-step blocks via host-computed fp64
block bases (exact cumulative angle at each 128-step boundary), so per-delta
rounding error amplifies by at most sqrt(128) and no cross-core communication
is needed.

Everything heavy runs on the PE in [time-partition, pair-free] layout:
    deltas block [128t, 256p] = sum_dc xT_chunk[128d,128t]^T @ W_chunk[128d,256p]
        (x^T chunks are the STATIONARY operand, fp16 single pass)
    angles = U^T @ fp16(deltas) + ones*base_hi/lo + ramp*b_hi/lo
        (U = upper-triangular ones: the per-block cumsum is ONE matmul;
         the rank-4 affine matmul injects the host base and the per-step
         bias t*b exactly — no DVE scan instructions at all)
    trig: magic-number range reduction to rs in [-0.5, 0.5] turns;
        sin = Sin(2pi*rs) on ScalarE; cos = Sin(pi/2 - 2pi*|rs|) reusing the
        SAME reduction (|rs| via one DVE abs_max) — 4 ScalarE passes total
    rotation on DVE in all-fp16 (2x perf mode), wide [128, 1024] tiles
        spanning 4 blocks to amortize instruction/init overheads
    out written fp16 (well within tolerance), host upcasts + passthrough.
"""

import sys

if "/opt/trn_rl_repo" not in sys.path:
    sys.path.insert(0, "/opt/trn_rl_repo")

from contextlib import ExitStack

import numpy as np

import concourse.bacc as bacc
import concourse.bass as bass
import concourse.mybir as mybir
import concourse.tile as tile
from concourse.bass_utils import run_bass_kernel_spmd

F32 = mybir.dt.float32
F16 = mybir.dt.float16
ADD = mybir.AluOpType.add
SUB = mybir.AluOpType.subtract
IDENT = mybir.ActivationFunctionType.Identity
SIN = mybir.ActivationFunctionType.Sin
ABS = mybir.ActivationFunctionType.Abs

D = 2048          # input feature dim (contraction)
P = 256           # delta-pairs dim
ROT = 2 * P       # rotated columns (512)
TL = 2048         # time steps per shard
BK = 128          # cumsum block (base injection granularity)
NBK = TL // BK    # blocks per shard (16)
G = 4             # blocks per group (wide-tile span)
NG = NBK // G     # groups per shard (4)
KC = D // 128     # contraction chunks (16)
WID = G * P       # wide-tile columns (1024)
N_CORES = 8

MAGIC = 12582912.0          # 1.5 * 2**23: fp32 round-to-int magic constant
SCALE_2PI = 6.28310         # slightly < 2*pi so Sin args stay inside [-pi, pi]
HALF_PI = 1.5707964


def build_program() -> bass.Bass:
    nc = bacc.Bacc("TRN2", target_bir_lowering=False, debug=False)

    # x^T tiles, row block g: [128 d-part, bkl*KC*128 + dc*128 + t_local]
    xt = nc.dram_tensor("xt", [NG * 128, G * KC * 128], F16,
                        kind="ExternalInput").ap()
    # W, d-chunks along free: [128 d-part, dc*P + p] fp16
    w = nc.dram_tensor("w", [128, KC * P], F16, kind="ExternalInput").ap()
    # upper-triangular ones (u[t, t'] = 1 iff t <= t')
    u = nc.dram_tensor("u", [128, 128], F16, kind="ExternalInput").ap()
    # affine stationary: rows [ones, ones, ramp(1..128), ramp]
    afs = nc.dram_tensor("afs", [4, 128], F16, kind="ExternalInput").ap()
    # affine moving: rows [base_hi[bk,p], base_lo, b_hi, b_lo], bk-major
    afm = nc.dram_tensor("afm", [4, NBK * P], F16, kind="ExternalInput").ap()
    # natural-layout rotation operands, row block g:
    # [t_local, half*WID + bkl*P + p] fp16
    x12 = nc.dram_tensor("x12", [NG * 128, 2 * WID], F16,
                         kind="ExternalInput").ap()
    # rotated output, same indexing as x12
    outT = nc.dram_tensor("outT", [NG * 128, 2 * WID], F16,
                          kind="ExternalOutput").ap()

    with tile.TileContext(nc) as tc, ExitStack() as ctx:
        const_pool = ctx.enter_context(tc.tile_pool(name="const", bufs=1))
        w_pool = ctx.enter_context(tc.tile_pool(name="w", bufs=1))
        xt_pool = ctx.enter_context(tc.tile_pool(name="xt", bufs=2))
        x12_pool = ctx.enter_context(tc.tile_pool(name="x12", bufs=2))
        dp_pool = ctx.enter_context(
            tc.tile_pool(name="dp_psum", bufs=2, space="PSUM"))
        ang_pool = ctx.enter_context(
            tc.tile_pool(name="ang_psum", bufs=2, space="PSUM"))
        d16_pool = ctx.enter_context(tc.tile_pool(name="d16", bufs=2))
        a32_pool = ctx.enter_context(tc.tile_pool(name="a32", bufs=2))
        trig_pool = ctx.enter_context(tc.tile_pool(name="trig", bufs=2))
        rot_pool = ctx.enter_context(tc.tile_pool(name="rot", bufs=2))
        out_pool = ctx.enter_context(tc.tile_pool(name="out", bufs=2))

        # Stationary/constant loads: one dense DMA each
        w_sb = w_pool.tile([128, KC * P], F16, tag="w")
        nc.sync.dma_start(w_sb[:], w[:])
        u_sb = const_pool.tile([128, 128], F16, tag="u")
        nc.sync.dma_start(u_sb[:], u[:])
        afs_sb = const_pool.tile([4, 128], F16, tag="afs")
        nc.sync.dma_start(afs_sb[:], afs[:])
        afm_sb = const_pool.tile([4, NBK * P], F16, tag="afm")
        nc.sync.dma_start(afm_sb[:], afm[:])
        magic_sb = const_pool.tile([128, 1], F32, tag="magic")
        nc.gpsimd.memset(magic_sb[:], MAGIC)
        hpi_sb = const_pool.tile([128, 1], F32, tag="hpi")
        nc.gpsimd.memset(hpi_sb[:], HALF_PI)

        def angle_and_rotate(g, d16, x12t):
            """Angle matmuls + trig + rotation + out DMA for group g.

            Issued one group late so the PE's in-order queue never stalls
            on the Act delta-copy: while Act produces d16(g), the PE is
            already streaming the delta matmuls of group g+1.
            """
            ang = ang_pool.tile([128, WID], F32, tag="ang")
            for bkl in range(G):
                bk = g * G + bkl
                sl = slice(bkl * P, (bkl + 1) * P)
                nc.tensor.matmul(ang[:, sl], u_sb[:], d16[:, sl],
                                 start=True, stop=False)
                nc.tensor.matmul(ang[:, sl], afs_sb[:],
                                 afm_sb[:, bk * P:(bk + 1) * P],
                                 start=False, stop=True)

            # range reduction (turns): rs = y - round(y) in [-0.5, 0.5]
            a_s = a32_pool.tile([128, WID], F32, tag="a_s")
            nc.scalar.activation(a_s[:], ang[:], IDENT,
                                 bias=magic_sb[:], scale=-1.0)
            rs = trig_pool.tile([128, WID], F16, tag="rs")
            nc.vector.scalar_tensor_tensor(rs[:], a_s[:], MAGIC, ang[:],
                                           op0=SUB, op1=ADD)
            sn = trig_pool.tile([128, WID], F16, tag="sn")
            nc.scalar.activation(sn[:], rs[:], SIN, scale=SCALE_2PI)
            # cos(2pi*y) = sin(pi/2 - 2pi*|rs|), same reduction
            ra = trig_pool.tile([128, WID], F16, tag="ra")
            nc.scalar.activation(ra[:], rs[:], ABS)
            cs = trig_pool.tile([128, WID], F16, tag="cs")
            nc.scalar.activation(cs[:], ra[:], SIN,
                                 scale=-SCALE_2PI, bias=hpi_sb[:])

            # rotation, all-fp16 on DVE
            x1 = x12t[:, 0:WID]
            x2 = x12t[:, WID:2 * WID]
            o = out_pool.tile([128, 2 * WID], F16, tag="o")
            t1 = rot_pool.tile([128, WID], F16, tag="t1")
            nc.vector.tensor_mul(t1[:], x1, cs[:])
            t2 = rot_pool.tile([128, WID], F16, tag="t2")
            nc.vector.tensor_mul(t2[:], x2, sn[:])
            nc.vector.tensor_sub(o[:, 0:WID], t1[:], t2[:])
            t3 = rot_pool.tile([128, WID], F16, tag="t3")
            nc.vector.tensor_mul(t3[:], x2, cs[:])
            t4 = rot_pool.tile([128, WID], F16, tag="t4")
            nc.vector.tensor_mul(t4[:], x1, sn[:])
            nc.vector.tensor_add(o[:, WID:2 * WID], t3[:], t4[:])

            nc.sync.dma_start(outT[g * 128:(g + 1) * 128, :], o[:])

        pend = None  # (g, d16, x12t) awaiting its angle stage
        for g in range(NG):
            x12t = x12_pool.tile([128, 2 * WID], F16, tag="x12")
            nc.sync.dma_start(x12t[:], x12[g * 128:(g + 1) * 128, :])
            # one [128, G*KC*128] tile per group; group 0's DMA is split
            # per block so the first delta matmuls can start early
            xtg = xt_pool.tile([128, G * KC * 128], F16, tag="xt")
            if g == 0:
                for bkl in range(G):
                    cs = slice(bkl * KC * 128, (bkl + 1) * KC * 128)
                    nc.sync.dma_start(xtg[:, cs], xt[0:128, cs])
            else:
                nc.sync.dma_start(xtg[:], xt[g * 128:(g + 1) * 128, :])

            # deltas^T for 4 blocks: [128 t, bkl*P + p] in PSUM
            dp = dp_pool.tile([128, WID], F32, tag="dp")
            for bkl in range(G):
                sl = slice(bkl * P, (bkl + 1) * P)
                for dc in range(KC):
                    nc.tensor.matmul(
                        dp[:, sl],
                        xtg[:, (bkl * KC + dc) * 128:(bkl * KC + dc + 1) * 128],
                        w_sb[:, dc * P:(dc + 1) * P],
                        start=(dc == 0), stop=(dc == KC - 1))

            # fp16 copy of deltas (moving operand of the cumsum matmul)
            d16 = d16_pool.tile([128, WID], F16, tag="d16")
            nc.scalar.activation(d16[:], dp[:], IDENT)

            if pend is not None:
                angle_and_rotate(*pend)
            pend = (g, d16, x12t)
        angle_and_rotate(*pend)

    nc.compile()
    return nc


_NC_CACHE: dict = {}


def _get_nc():
    if "nc" not in _NC_CACHE:
        _NC_CACHE["nc"] = build_program()
    return _NC_CACHE["nc"]


def prepare_weights(W: np.ndarray, b: np.ndarray):
    inv2pi = 1.0 / (2.0 * np.pi)
    Wt = W.astype(np.float64).T * inv2pi                       # [D, P]
    wh = Wt.astype(np.float16)
    bt = b.astype(np.float64) * inv2pi                         # [P]
    bh = bt.astype(np.float16)
    bl = (bt - bh.astype(np.float64)).astype(np.float16)
    # [D, P] -> [128, KC*P] with d-chunks along the free dim
    w_in = np.ascontiguousarray(
        wh.reshape(KC, 128, P).transpose(1, 0, 2).reshape(128, KC * P))
    # Bases must come from the FULL-precision weights so each 128-step block
    # restarts at the reference-exact angle: the device's fp16-W error then
    # only drifts within one block instead of accumulating across the shard.
    return w_in, bh, bl, Wt, bt


def make_in_maps(x: np.ndarray, W: np.ndarray, b: np.ndarray):
    B, T, _ = x.shape
    w_in, bh, bl, w_eff, b_eff = prepare_weights(W, b)

    u_in = np.triu(np.ones((128, 128), np.float16))
    afs_in = np.stack([
        np.ones(128, np.float16), np.ones(128, np.float16),
        np.arange(1, 129, dtype=np.float16),
        np.arange(1, 129, dtype=np.float16)])

    # fp64 cumulative angle at every 128-step boundary, per batch (turns)
    nblk = T // BK                                              # 32
    xblk = x.reshape(B, nblk, BK, D).sum(axis=2, dtype=np.float64)
    dblk = xblk @ w_eff + BK * b_eff                            # [B, 32, P]
    bases = np.zeros((B, nblk, P))
    np.cumsum(dblk[:, :-1], axis=1, out=bases[:, 1:])           # exclusive

    in_maps = []
    for c in range(N_CORES):
        bb, hh = c // 2, c % 2
        xs = x[bb, hh * TL:(hh + 1) * TL, :].astype(np.float16)  # [TL, D]
        # xt: [g*128 + dp, (bkl*KC + dc)*128 + tl] = xs[(g*G+bkl)*128+tl,
        #                                              dc*128 + dp]
        xt_in = np.ascontiguousarray(
            xs.reshape(NG, G, BK, KC, 128).transpose(0, 4, 1, 3, 2)
            .reshape(NG * 128, G * KC * 128))
        # x12: [g*128 + tl, half*WID + bkl*P + p]
        x12_in = np.ascontiguousarray(
            xs[:, :ROT].reshape(NG, G, BK, 2, P).transpose(0, 2, 3, 1, 4)
            .reshape(NG * 128, 2 * WID))
        bs = bases[bb, hh * NBK:(hh + 1) * NBK]                 # [NBK, P]
        bs_hi = bs.astype(np.float16)
        bs_lo = (bs - bs_hi.astype(np.float64)).astype(np.float16)
        afm_in = np.stack([
            bs_hi.reshape(NBK * P), bs_lo.reshape(NBK * P),
            np.tile(bh, NBK), np.tile(bl, NBK)])
        in_maps.append({
            "xt": xt_in, "w": w_in, "u": u_in,
            "afs": afs_in, "afm": np.ascontiguousarray(afm_in),
            "x12": x12_in,
        })
    return in_maps


def assemble_output(x: np.ndarray, results) -> np.ndarray:
    B, T, Din = x.shape
    out = np.empty((B, T, Din), np.float32)
    out[:, :, ROT:] = x[:, :, ROT:]
    for c in range(N_CORES):
        bb, hh = c // 2, c % 2
        r = results[c]["outT"]                                  # [NG*128, 2*WID]
        blk = (r.reshape(NG, BK, 2, G, P).transpose(0, 3, 1, 2, 4)
               .reshape(TL, ROT))
        out[bb, hh * TL:(hh + 1) * TL, :ROT] = blk.astype(np.float32)
    return out


def kernel(x: np.ndarray, W: np.ndarray, b: np.ndarray) -> np.ndarray:
    nc = _get_nc()
    in_maps = make_in_maps(x, W, b)
    res = run_bass_kernel_spmd(nc, in_maps, list(range(N_CORES)))
    return assemble_output(x, res.results)
